# revision 42
# baseline (speedup 1.0000x reference)
"""Causal single-head attention on 8 Trainium2 NeuronCores.

Problem: x[8, 2048, 1024] -> out[8, 2048, 64]
  q/k/v = x @ W{q,k,v} + b{q,k,v};  out = softmax(causal(q k^T / 8)) v

Sharding: data-parallel over batch; core b computes batch element b.

Per-core design (T=2048, D=1024, H=64), quarter-pipelined:
  - host pre-packs x[b]^T quarter-major xq [4, 128p, 8c, 512t] so each
    512-wide t-quarter lands as 128 contiguous 2KB-descriptor DMAs;
    weights pre-arranged [128p, c, m] on host.
  - per quarter iq: QK projection (lhsT=[Wq|Wk] chunk, full PE array)
    -> psum [128(qh|kh), 512]; bias-add copies into qT2/kT2 [128, T]
    with q (resp. k) DUPLICATED in both partition halves, enabling
    row-tiled attention matmuls.  V projection col-tiled 2x: even
    chunks -> psum[0:64], odd chunks -> psum[64:128] concurrently
    (independent PE column groups), summed + biased into vT (bf16),
    then PE-transposed (bf16, 1 cyc/col) into natural v_sb tiles
    [128t, 64h | 1 | 0].
  - attention for i-quarter iq over j-chunk PAIRS (jtA=2r, jtB=2r+1):
    S^T pair psum [128, LA+LB]: tile A = kT2[0:64].T @ qT2[0:64] in PE
    row-group 0, tile B in row-group 1 -> the two 128x(<=512) matmuls
    run CONCURRENTLY; ONE merged exp ACT over [0:LA+LB] (scale=1/8)
    halves the ACT fixed cost; causal diagonal 128-blocks masked by
    gpsimd affine_select; PV accumulates ps_o [66, 512] += [v|1|0].T@P
    (row 64 = softmax denominator).  Software pipeline: S(r+1) is
    emitted before PV(r) so the PE works while ACT computes exp(r).
  - QKV(q+1) matmuls are interleaved into attn(q) rounds as fillers to
    keep the PE warm and overlap the x-quarter DMA.
  - drain per quarter: psum -> oT bf16, PE-transpose 66x128 blocks,
    divide by denominator, DMA out to packed [4, 128p, 4tt, 64] dram;
    host unpermutes.
"""

import os
from contextlib import ExitStack

import ml_dtypes
import numpy as np

import concourse.bacc as bacc
import concourse.mybir as mybir
import concourse.tile as tile
from concourse.bass_utils import run_bass_kernel_spmd

F32 = mybir.dt.float32
F32R = mybir.dt.float32r
BF16 = mybir.dt.bfloat16
AF = mybir.ActivationFunctionType
ALU = mybir.AluOpType

T = 2048
D = 1024
H = 64
NB = 8
DC = D // 128      # 8 contraction chunks
NJT = T // 128     # 16 j-chunks (also 16 t-tiles)
QW = 512           # i-quarter width
NQ = T // QW       # 4 quarters
SCALE = 1.0 / 8.0  # 1/sqrt(H)

_CACHE: dict = {}


class Ctxt:
    pass


def _emit_qk_mm(g, q):
    ps_qk = g.pss.tile([128, QW], F32, tag="s", name=f"psqk{q}")
    for c in range(DC):
        g.nc.tensor.matmul(
            ps_qk[:], g.wqkv_sb[:, c, 0:128], g.xt_sb[:, q, c, :],
            start=(c == 0), stop=(c == DC - 1),
        )
    return ps_qk


def _emit_qk_copy(g, q, ps_qk):
    nc = g.nc
    sl = slice(q * QW, (q + 1) * QW)
    # qkT_a = [q; k] straight full-lane copy; qkT_b = [k; q] swapped
    # halves.  S-tile A reads k from qkT_b[0:64] / q from qkT_a[0:64];
    # S-tile B reads k from qkT_a[64:128] / q from qkT_b[64:128].
    nc.vector.tensor_scalar(
        out=g.qkT_a[:, sl], in0=ps_qk[:, :],
        scalar1=g.bqk_sb[:], scalar2=None, op0=ALU.add,
    )
    nc.vector.tensor_copy(g.qkT_b[0:64, sl], g.qkT_a[64:128, sl])
    nc.vector.tensor_copy(g.qkT_b[64:128, sl], g.qkT_a[0:64, sl])


def _emit_qk(g, q):
    _emit_qk_copy(g, q, _emit_qk_mm(g, q))


def _emit_v(g, q):
    """V projection for quarter q, col-tiled 2x -> vT (bf16)."""
    nc = g.nc
    ps_v = g.pss.tile([128, QW], F32, tag="s", name=f"psv{q}")
    # Pre-clear: one zero matmul writes the whole bank with start=True,
    # setting every has_written bit.  The two concurrently-executing
    # col-tiled groups below then all run with start=False (accumulate),
    # so neither can zap the other's partial sums with a bank-wide
    # has_written clear (the race that corrupted vT before).
    nc.tensor.matmul(ps_v[:], g.warm[:, 0:128], g.warm[:],
                     start=True, stop=False)
    for cp in range(DC // 2):
        nc.tensor.matmul(
            ps_v[0:64, :], g.wqkv_sb[:, 2 * cp, 128:192],
            g.xt_sb[:, q, 2 * cp, :],
            start=False, stop=False,
        )
        nc.tensor.matmul(
            ps_v[64:128, :], g.wqkv_sb[:, 2 * cp + 1, 128:192],
            g.xt_sb[:, q, 2 * cp + 1, :],
            start=False, stop=(cp == DC // 2 - 1),
        )
    sl = slice(q * QW, (q + 1) * QW)
    nc.vector.tensor_scalar(
        out=g.vT[:, sl], in0=ps_v[0:64, :],
        scalar1=g.bv_sb[:], scalar2=None, op0=ALU.add,
    )
    nc.vector.tensor_tensor(
        out=g.vT[:, sl], in0=g.vT[:, sl], in1=ps_v[64:128, :],
        op=ALU.add,
    )


def _emit_vt(g, q):
    """Transpose quarter q of v^T into natural v_sb tiles (bf16 PE)."""
    nc = g.nc
    ps_t = g.pss.tile([128, 4, 128], F32, tag="s", name=f"pst{q}")
    for j2 in range(4):
        jt = q * 4 + j2
        nc.tensor.transpose(
            ps_t[:, j2, 0:H].bitcast(F32R),
            g.vT[:, jt * 128:(jt + 1) * 128],
            g.ident[0:64, 0:64],
        )
    nc.vector.tensor_copy(g.v_sb[:, q * 4:q * 4 + 4, 0:H], ps_t[:, :, 0:H])


def _emit_s_pair(g, q, r):
    """Row-tiled S^T pair for round r of quarter q -> (pair_psum, geom)."""
    nc = g.nc
    w0 = q * QW
    jtA, jtB = 2 * r, 2 * r + 1
    offA = max(128 * jtA - w0, 0)
    offB = max(128 * jtB - w0, 0)
    LA, LB = QW - offA, QW - offB
    ps_s = g.ps.tile([128, 2 * QW], F32, tag="w", name=f"s{q}_{r}")
    # A in bank 0 at [0:LA]; B ALWAYS in bank 1 at [QW:QW+LB]: the two
    # row-tiled matmuls run concurrently and must not drain into the
    # same PSUM bank.  exp later covers [0:QW+LB]; the [LA:QW] gap is
    # stale-but-bounded psum, never read by PV.
    nc.tensor.matmul(
        ps_s[:, 0:LA],
        g.qkT_b[0:64, jtA * 128:(jtA + 1) * 128],
        g.qkT_a[0:64, w0 + offA:w0 + QW],
        start=True, stop=True,
    )
    nc.tensor.matmul(
        ps_s[:, QW:QW + LB],
        g.qkT_a[64:128, jtB * 128:(jtB + 1) * 128],
        g.qkT_b[64:128, w0 + offB:w0 + QW],
        start=True, stop=True,
    )
    return ps_s, (offA, offB, LA, LB)


def _emit_exp(g, q, r, ps_s, geom):
    """Merged exp for the pair + diagonal masks; returns P tile."""
    nc = g.nc
    offA, offB, LA, LB = geom
    P = g.ppool.tile([128, 2 * QW], BF16, tag="P")
    nc.scalar.activation(
        out=P[:, 0:QW + LB], in_=ps_s[:, 0:QW + LB], func=AF.Exp,
        scale=SCALE,
    )
    if 128 * (2 * r) >= q * QW:  # diagonal pair (r in {2q, 2q+1})
        nc.gpsimd.affine_select(
            out=P[:, 0:128], in_=P[:, 0:128],
            compare_op=ALU.is_ge, fill=0.0,
            base=0, pattern=[[1, 128]], channel_multiplier=-1,
        )
        nc.gpsimd.affine_select(
            out=P[:, QW:QW + 128], in_=P[:, QW:QW + 128],
            compare_op=ALU.is_ge, fill=0.0,
            base=0, pattern=[[1, 128]], channel_multiplier=-1,
        )
    return P


def _emit_pv(g, q, r, P, geom, ps_o, rmax, diag):
    nc = g.nc
    offA, offB, LA, LB = geom
    jtA, jtB = 2 * r, 2 * r + 1
    if diag and LA > 128:
        # non-diag columns don't wait for the affine_select masks
        nc.tensor.matmul(
            ps_o[:, offA + 128:QW], g.v_sb[:, jtA, :], P[:, 128:LA],
            start=(r == 0), stop=False,
        )
        nc.tensor.matmul(
            ps_o[:, offA:offA + 128], g.v_sb[:, jtA, :], P[:, 0:128],
            start=False, stop=False,
        )
    else:
        nc.tensor.matmul(
            ps_o[:, offA:QW], g.v_sb[:, jtA, :], P[:, 0:LA],
            start=(r == 0), stop=False,
        )
    if diag and LB > 128:
        nc.tensor.matmul(
            ps_o[:, offB + 128:QW], g.v_sb[:, jtB, :],
            P[:, QW + 128:QW + LB],
            start=False, stop=False,
        )
        nc.tensor.matmul(
            ps_o[:, offB:offB + 128], g.v_sb[:, jtB, :], P[:, QW:QW + 128],
            start=False, stop=(r == rmax),
        )
    else:
        nc.tensor.matmul(
            ps_o[:, offB:QW], g.v_sb[:, jtB, :], P[:, QW:QW + LB],
            start=False, stop=(r == rmax),
        )


def _emit_attn_quarter(g, q, fillers):
    """Software-pipelined attention rounds for i-quarter q."""
    nc = g.nc
    R = 2 * q + 2
    ps_o = g.out_ps.tile([66, QW], F32, tag="out", name=f"o{q}")
    ps_s, geom = _emit_s_pair(g, q, 0)
    pend = (ps_s, geom)
    for r in range(R):
        ps_s, geom = pend
        P = _emit_exp(g, q, r, ps_s, geom)
        if r + 1 < R:
            pend = _emit_s_pair(g, q, r + 1)
        if fillers:
            fillers.pop(0)()
        _emit_pv(g, q, r, P, geom, ps_o, R - 1, r >= 2 * q)
    return ps_o


def _drain_closures(g, q, ps_o):
    """Column-halved drain as filler closures: [half0, half1, dma]."""
    nc = g.nc
    st = {}

    def _half(h2):
        def go():
            if "oT" not in st:
                st["oT"] = g.otpool.tile([66, QW], BF16, tag="oT",
                                         name=f"oT{q}")
                st["ps_n"] = g.pss.tile([128, 4, 256], BF16, tag="s",
                                        name=f"psn{q}")
            oT, ps_n = st["oT"], st["ps_n"]
            cs = slice(h2 * 256, h2 * 256 + 256)
            nc.vector.tensor_copy(oT[:, cs], ps_o[:, cs])
            for t2 in (2 * h2, 2 * h2 + 1):
                nc.tensor.transpose(
                    ps_n[:, t2, 0:66],
                    oT[:, t2 * 128:(t2 + 1) * 128],
                    g.identb[:, :],
                )
            sl2 = slice(q * 4 + 2 * h2, q * 4 + 2 * h2 + 2)
            nc.vector.tensor_copy(g.out_nat[:, sl2, :],
                                  ps_n[:, 2 * h2:2 * h2 + 2, 0:H])
            nc.vector.reciprocal(g.recip[:, sl2],
                                 ps_n[:, 2 * h2:2 * h2 + 2, H])
            for tt in range(q * 4 + 2 * h2, q * 4 + 2 * h2 + 2):
                nc.vector.tensor_scalar_mul(
                    g.out_nat[:, tt, 0:H], g.out_nat[:, tt, 0:H],
                    g.recip[:, tt:tt + 1])
        return go

    def _dma():
        sl = slice(q * 4, (q + 1) * 4)
        for p0 in range(0, 128, 32):
            nc.sync.dma_start(out=g.out[q, p0:p0 + 32],
                              in_=g.out_nat[p0:p0 + 32, sl, 0:H])

    return [_half(0), _half(1), _dma]


def _emit_drain(g, q, ps_o):
    for go in _drain_closures(g, q, ps_o):
        go()


def _build():
    nc = bacc.Bacc("TRN2", target_bir_lowering=False, debug=False,
                   num_devices=NB)
    g = Ctxt()
    g.nc = nc
    xq = nc.dram_tensor("xq", [NQ, 128, DC, QW], BF16, kind="ExternalInput")
    wqkv = nc.dram_tensor("wqkv", [128, DC, 128 + H], BF16,
                          kind="ExternalInput")
    cblob = nc.dram_tensor("cblob", [128, 8], mybir.dt.uint8,
                           kind="ExternalInput")
    identd = nc.dram_tensor("identd", [66, 66], F32R, kind="ExternalInput")
    identbd = nc.dram_tensor("identbd", [66, 66], BF16, kind="ExternalInput")
    g.out = nc.dram_tensor("out", [NQ, 128, 4, H], F32,
                           kind="ExternalOutput")

    with ExitStack() as ctx:
        tc = ctx.enter_context(tile.TileContext(nc))
        const = ctx.enter_context(tc.tile_pool(name="const", bufs=1))
        big = ctx.enter_context(tc.tile_pool(name="big", bufs=1))
        g.ppool = ctx.enter_context(tc.tile_pool(name="ppool", bufs=4))
        g.otpool = ctx.enter_context(tc.tile_pool(name="otpool", bufs=2))
        g.ps = ctx.enter_context(tc.tile_pool(name="ps", bufs=2,
                                              space="PSUM"))
        g.pss = ctx.enter_context(tc.tile_pool(name="pss", bufs=2,
                                               space="PSUM"))
        g.out_ps = ctx.enter_context(
            tc.tile_pool(name="out_ps", bufs=2, space="PSUM"))

        # DMA kick order is engineered for the 16 round-robin queues:
        # kick #0 = weights (queue 0), #1-8 = x quarter 0 single-chunk
        # kicks (queues 1-8), #9-15 = small constants (queues 9-15),
        # #16-23 = quarter 1 (queues 0-7, BEHIND weights/q0 pieces),
        # #24-31 = quarter 2 (queues 8-15), #32-39 = quarter 3 (0-7).
        # Each queue drains in order, so quarters arrive staggered
        # q0 -> q1 -> q2 -> q3 instead of all finishing together.
        g.xt_sb = big.tile([128, NQ, DC, QW], BF16)
        g.wqkv_sb = const.tile([128, DC, 128 + H], BF16)
        g.cb = const.tile([128, 8], mybir.dt.uint8)
        g.bqk_sb = g.cb[:, 0:4].bitcast(F32)
        g.bv_sb = g.cb[0:64, 4:8].bitcast(F32)
        g.ident = const.tile([66, 66], F32R)
        g.identb = const.tile([66, 66], BF16)

        def kick_xq(q, step):
            for c0 in range(0, DC, step):
                nc.sync.dma_start(
                    out=g.xt_sb[:, q, c0:c0 + step, :],
                    in_=xq[q, :, c0:c0 + step, :])

        nc.sync.dma_start(out=g.wqkv_sb[:], in_=wqkv[:])
        kick_xq(0, 2)
        nc.sync.dma_start(out=g.cb[:], in_=cblob[:])
        nc.sync.dma_start(out=g.ident[:], in_=identd[:])
        nc.sync.dma_start(out=g.identb[:], in_=identbd[:])

        g.qkT_a = big.tile([128, T], BF16)
        g.qkT_b = big.tile([128, T], BF16)
        g.vT = big.tile([64, T], F32R)
        g.v_sb = big.tile([128, NJT, H + 2], BF16)
        nc.vector.memset(g.v_sb[:, :, H:H + 1], 1.0)
        nc.vector.memset(g.v_sb[:, :, H + 1:H + 2], 0.0)
        for q in range(1, NQ):
            kick_xq(q, 4)
        g.out_nat = big.tile([128, NJT, H], F32)
        g.recip = const.tile([128, NJT], F32)

        # PE warmup + ACT table preload during the input-DMA window.
        warm = const.tile([128, 512], BF16)
        g.warm = warm
        nc.vector.memset(warm[:], 0.0)
        escr = const.tile([128, 2], F32)
        nc.vector.memset(escr[:], 0.0)
        nc.scalar.activation(
            out=escr[:], in_=escr[:], func=AF.Exp, scale=1.0,
        )
        ps_w = g.pss.tile([128, 512], F32, tag="s")
        for _ in range(8):
            nc.tensor.matmul(ps_w[:], warm[:, 0:128], warm[:],
                             start=True, stop=True)

        def qkv_fillers(q):
            st = {}

            def qk_mm():
                st["ps"] = _emit_qk_mm(g, q)

            return [
                qk_mm,
                lambda: _emit_qk_copy(g, q, st["ps"]),
                lambda: _emit_v(g, q),
                lambda: _emit_vt(g, q),
            ]

        # quarter pipeline; drains of earlier quarters become fillers in
        # later quarters' rounds so no serial drain blob sits at the tail
        for go in qkv_fillers(0):
            go()
        fillers = qkv_fillers(1)
        ps_os = {}
        for q in range(NQ):
            ps_os[q] = _emit_attn_quarter(g, q, fillers)
            for go in fillers:   # leftover QKV work must precede attn(q+1)
                go()
            fillers = qkv_fillers(q + 2) if q + 2 < NQ else []
            if q == 1:
                fillers += _drain_closures(g, 0, ps_os[0])
            elif q == 2:
                fillers += (_drain_closures(g, 1, ps_os[1])
                            + _drain_closures(g, 2, ps_os[2]))
        _emit_drain(g, 3, ps_os[3])

    nc.compile()
    return nc


def _get_nc():
    if "nc" not in _CACHE:
        _CACHE["nc"] = _build()
    return _CACHE["nc"]


def kernel(x, Wq, bq, Wk, bk, Wv, bv):
    x = np.ascontiguousarray(np.asarray(x, dtype=np.float32))
    Wq = np.asarray(Wq, dtype=np.float32)
    Wk = np.asarray(Wk, dtype=np.float32)
    Wv = np.ascontiguousarray(np.asarray(Wv, dtype=np.float32))
    bq = np.asarray(bq, dtype=np.float32)
    bk = np.asarray(bk, dtype=np.float32)
    bv = np.asarray(bv, dtype=np.float32)

    wqkv = np.concatenate([Wq, Wk, Wv], axis=1).astype(ml_dtypes.bfloat16)
    wqkv_p = np.ascontiguousarray(
        wqkv.reshape(DC, 128, 128 + H).transpose(1, 0, 2))
    x_b = x.astype(ml_dtypes.bfloat16)
    cblob = np.zeros((128, 8), dtype=np.uint8)
    cblob[:, 0:4] = np.concatenate([bq, bk]).astype(np.float32)[:, None].view(np.uint8)
    cblob[0:64, 4:8] = bv.astype(np.float32)[:, None].view(np.uint8)
    identb = np.eye(66, dtype=np.float32)
    identb16 = np.eye(66, dtype=ml_dtypes.bfloat16)

    in_maps = []
    for b in range(NB):
        xt = x_b[b].T  # [D, T]
        xqp = np.ascontiguousarray(
            xt.reshape(DC, 128, NQ, QW).transpose(2, 1, 0, 3))
        in_maps.append({
            "xq": xqp,
            "wqkv": wqkv_p,
            "cblob": cblob,
            "identd": identb,
            "identbd": identb16,
        })

    nc = _get_nc()
    trace = bool(int(os.environ.get("KTRACE", "0")))
    res = run_bass_kernel_spmd(
        nc, in_maps, core_ids=list(range(NB)), trace=trace,
    )
    if trace:
        _CACHE["exec_time_ns"] = res.exec_time_ns
        _CACHE["results"] = res
    outs = []
    for r in res.results:
        o = r["out"]  # [NQ, 128, 4, H]; t = q*512 + tt*128 + p
        outs.append(o.transpose(0, 2, 1, 3).reshape(T, H))
    return np.stack(outs)
